# revision 3
# baseline (speedup 1.0000x reference)
"""Trainium2 Bass kernel for a fused LSTM cell.

Problem: B=8192, I=H=1024.
  gates = [x, h_prev] @ [W_f|W_i|W_o|W_C] + b      # [B, 4H]
  C_t = sigmoid(f)*C_prev + sigmoid(i)*tanh(c)
  h_t = sigmoid(o)*tanh(C_t)

Sharding: data-parallel over batch across 8 NeuronCores (1024 rows each),
weights replicated. No collectives needed.

Per-core device program (fp32r matmuls — full PE rate, ~1e-4 rel err):
  - combined^T (K=2048 x M=1024) resident in SBUF, K on partitions.
  - W streamed as [128, 16, 128] tiles (K-chunk x N-chunk), hidden dim on
    PSUM partitions so the per-gate bias rides the ScalarE activation's
    per-partition bias operand: gate = act(psum + b).
  - Loop q (8 H-chunks of 128) x m (2 batch-chunks of 512): 4 gates x 16
    K-chunks of matmuls into 4 PSUM banks, ScalarE sigmoid/tanh eviction,
    VectorE elementwise for C_t / h_t, DMA out in [H, B] layout
    (untransposed on host).

All host-side layout shuffles (transpose/concat/reorder) are numpy copies
outside the measured device execution.
"""

import numpy as np

import concourse.bass as bass
import concourse.mybir as mybir
import concourse.tile as tile
from concourse import bacc
from concourse.bass_utils import run_bass_kernel_spmd

N_CORES = 8
B, I, H = 8192, 1024, 1024
K = I + H                      # 2048 contraction dim
BL = B // N_CORES              # 1024 batch rows per core
KC = K // 128                  # 16 K-chunks
QC = H // 128                  # 8 hidden chunks of 128
MC = 2                         # batch chunks of 512 per core
MT = BL // MC                  # 512
NCHUNKS = 4 * QC               # 32 (q-major, gate-minor) N-chunks of 128

_DT_MM = mybir.dt.float32r     # matmul operand dtype (fp32 bits, fast path)

_SIG = mybir.ActivationFunctionType.Sigmoid
_TANH = mybir.ActivationFunctionType.Tanh


def build_program(repeats: int = 1):
    """Build the per-core Bass program. `repeats` unrolls the whole body
    (same data) for slope-based HW timing in test harnesses."""
    nc = bacc.Bacc("TRN2", target_bir_lowering=False, debug=False)

    # Host-prepped layouts (see prep_inputs):
    #   comb: [128, KC, BL]   combined^T, partition-major contiguous
    #   w:    [NCHUNKS, 128, KC, 128]  W tiles, partition-major contiguous
    #   bt:   [128, NCHUNKS]  bias chunks
    #   cp:   [128, QC, BL]   C_prev^T
    comb_d = nc.dram_tensor("comb", [128, KC, BL], _DT_MM, kind="ExternalInput")
    w_d = nc.dram_tensor("w", [NCHUNKS, 128, KC, 128], _DT_MM, kind="ExternalInput")
    bt_d = nc.dram_tensor("bt", [128, NCHUNKS], mybir.dt.float32, kind="ExternalInput")
    cp_d = nc.dram_tensor("cp", [128, QC, BL], mybir.dt.float32, kind="ExternalInput")
    ht_d = nc.dram_tensor("ht", [QC, 128, BL], mybir.dt.float32, kind="ExternalOutput")
    ct_d = nc.dram_tensor("ct", [QC, 128, BL], mybir.dt.float32, kind="ExternalOutput")

    with tile.TileContext(nc) as tc:
        with (
            tc.tile_pool(name="res", bufs=1) as res,
            tc.tile_pool(name="wp", bufs=6) as wp,
            tc.tile_pool(name="gp", bufs=2) as gp,
            tc.tile_pool(name="ep", bufs=2) as ep,
            tc.tile_pool(name="psum", bufs=2, space="PSUM") as pp,
        ):
            ct_sb = res.tile([128, KC, BL], _DT_MM)
            nc.sync.dma_start(out=ct_sb[:], in_=comb_d.ap())
            bt_sb = res.tile([128, NCHUNKS], mybir.dt.float32)
            nc.sync.dma_start(out=bt_sb[:], in_=bt_d.ap())
            cp_sb = res.tile([128, QC, BL], mybir.dt.float32)
            nc.sync.dma_start(out=cp_sb[:], in_=cp_d.ap())

            for _ in range(repeats):
                for q in range(QC):
                    wts = []
                    for g in range(4):
                        c = q * 4 + g
                        wt = wp.tile([128, KC, 128], _DT_MM, tag="wt")
                        nc.sync.dma_start(out=wt[:], in_=w_d.ap()[c])
                        wts.append(wt)
                    for m in range(MC):
                        ms = slice(m * MT, (m + 1) * MT)
                        ps = [
                            pp.tile([128, MT], mybir.dt.float32, name=f"ps{g}", tag=f"ps{g}")
                            for g in range(4)
                        ]
                        for g in range(4):
                            for k in range(KC):
                                nc.tensor.matmul(
                                    ps[g][:],
                                    lhsT=wts[g][:, k, :],
                                    rhs=ct_sb[:, k, ms],
                                    start=(k == 0),
                                    stop=(k == KC - 1),
                                )
                        # gate activations with fused per-partition bias
                        f_sb = gp.tile([128, MT], mybir.dt.float32, tag="f")
                        i_sb = gp.tile([128, MT], mybir.dt.float32, tag="i")
                        o_sb = gp.tile([128, MT], mybir.dt.float32, tag="o")
                        cl_sb = gp.tile([128, MT], mybir.dt.float32, tag="cl")
                        c0 = q * 4
                        nc.scalar.activation(f_sb[:], ps[0][:], _SIG, bias=bt_sb[:, c0 : c0 + 1])
                        nc.scalar.activation(i_sb[:], ps[1][:], _SIG, bias=bt_sb[:, c0 + 1 : c0 + 2])
                        nc.scalar.activation(o_sb[:], ps[2][:], _SIG, bias=bt_sb[:, c0 + 2 : c0 + 3])
                        nc.scalar.activation(cl_sb[:], ps[3][:], _TANH, bias=bt_sb[:, c0 + 3 : c0 + 4])
                        # C_t = f*C_prev + i*ctilda ; h_t = o*tanh(C_t)
                        t1 = ep.tile([128, MT], mybir.dt.float32, tag="t1")
                        t2 = ep.tile([128, MT], mybir.dt.float32, tag="t2")
                        c_out = ep.tile([128, MT], mybir.dt.float32, tag="c_out")
                        th = ep.tile([128, MT], mybir.dt.float32, tag="th")
                        h_out = ep.tile([128, MT], mybir.dt.float32, tag="h_out")
                        nc.vector.tensor_tensor(
                            t1[:], f_sb[:], cp_sb[:, q, ms], mybir.AluOpType.mult
                        )
                        nc.vector.tensor_tensor(
                            t2[:], i_sb[:], cl_sb[:], mybir.AluOpType.mult
                        )
                        nc.vector.tensor_tensor(
                            c_out[:], t1[:], t2[:], mybir.AluOpType.add
                        )
                        nc.scalar.activation(th[:], c_out[:], _TANH)
                        nc.vector.tensor_tensor(
                            h_out[:], o_sb[:], th[:], mybir.AluOpType.mult
                        )
                        nc.sync.dma_start(out=ct_d.ap()[q, :, ms], in_=c_out[:])
                        nc.sync.dma_start(out=ht_d.ap()[q, :, ms], in_=h_out[:])
    nc.compile()
    return nc


def prep_inputs(x, h_prev, C_prev, W_f, b_f, W_i, b_i, W_C, b_C, W_o, b_o):
    """Shard + lay out host arrays for the device program. Returns in_maps."""
    f32 = np.float32
    x = np.ascontiguousarray(x, f32)
    h_prev = np.ascontiguousarray(h_prev, f32)
    C_prev = np.ascontiguousarray(C_prev, f32)

    # W tiles: w5[c, p, ko, n] = W_gate[ko*128+p, q*128+n], c = q*4+g
    # Build as [QC, 4, 128(p), KC, 128(n)] then reshape.
    w5 = np.empty((QC, 4, 128, KC, 128), f32)
    for g, Wg in enumerate((W_f, W_i, W_o, W_C)):
        Wg = np.ascontiguousarray(Wg, f32)
        # [K, H] -> [KC, 128(p), QC, 128(n)] -> (q, p, ko, n)
        wr = Wg.reshape(KC, 128, QC, 128)
        w5[:, g] = wr.transpose(2, 1, 0, 3)
    w5 = np.ascontiguousarray(w5.reshape(NCHUNKS, 128, KC, 128))

    bt = np.empty((QC, 4, 128), f32)
    for g, bg in enumerate((b_f, b_i, b_o, b_C)):
        bt[:, g] = np.asarray(bg, f32).reshape(QC, 128)
    bt = np.ascontiguousarray(bt.reshape(NCHUNKS, 128).T)  # [128, NCHUNKS]

    in_maps = []
    for c in range(N_CORES):
        rs = slice(c * BL, (c + 1) * BL)
        # combined^T: [128(p), KC, BL]; rows 0..I-1 = x^T, I..K-1 = h^T
        comb = np.empty((KC, 128, BL), f32)
        comb.reshape(K, BL)[:I] = x[rs].T
        comb.reshape(K, BL)[I:] = h_prev[rs].T
        comb = np.ascontiguousarray(comb.transpose(1, 0, 2))
        # C_prev^T: [128(p), QC, BL]
        cp = np.ascontiguousarray(
            C_prev[rs].T.reshape(QC, 128, BL).transpose(1, 0, 2)
        )
        in_maps.append({"comb": comb, "w": w5, "bt": bt, "cp": cp})
    return in_maps


def assemble_outputs(results):
    """Gather per-core [QC, 128, BL] outputs into full [B, H] h_t, C_t."""
    h_t = np.empty((B, H), np.float32)
    C_t = np.empty((B, H), np.float32)
    for c, r in enumerate(results):
        rs = slice(c * BL, (c + 1) * BL)
        # [QC, 128, BL] -> [BL, QC*128]
        h_t[rs] = r["ht"].reshape(H, BL).T
        C_t[rs] = r["ct"].reshape(H, BL).T
    return h_t, C_t


_NC_CACHE = {}


def kernel(**inputs):
    if "nc" not in _NC_CACHE:
        _NC_CACHE["nc"] = build_program(repeats=1)
    nc = _NC_CACHE["nc"]
    in_maps = prep_inputs(**inputs)
    res = run_bass_kernel_spmd(nc, in_maps, core_ids=list(range(N_CORES)))
    return assemble_outputs(res.results)


# revision 13
# speedup vs baseline: 23.0761x; 23.0761x over previous
"""Trainium2 Bass kernel for a fused LSTM cell.

Problem: B=8192, I=H=1024.
  gates = [x, h_prev] @ [W_f|W_i|W_o|W_C] + b      # [B, 4H]
  C_t = sigmoid(f)*C_prev + sigmoid(i)*tanh(c)
  h_t = sigmoid(o)*tanh(C_t)

Sharding: data-parallel over batch across 8 NeuronCores (1024 rows each),
weights replicated. No collectives needed.

Per-core device program (fp32r matmuls — full PE rate, ~1e-4 rel err):
  - combined^T (K=2048 x M=1024) resident in SBUF, K on partitions.
  - W streamed as [128, 16, 128] tiles (K-chunk x N-chunk), hidden dim on
    PSUM partitions so the per-gate bias rides the ScalarE activation's
    per-partition bias operand: gate = act(psum + b).
  - Loop q (8 H-chunks of 128) x m (2 batch-chunks of 512): 4 gates x 16
    K-chunks of matmuls into 4 PSUM banks, ScalarE sigmoid/tanh eviction,
    VectorE elementwise for C_t / h_t, DMA out in [H, B] layout
    (untransposed on host).

All host-side layout shuffles (transpose/concat/reorder) are numpy copies
outside the measured device execution.
"""

import numpy as np

import concourse.bass as bass
import concourse.mybir as mybir
import concourse.tile as tile
from concourse import bacc
from concourse.bass_utils import run_bass_kernel_spmd

N_CORES = 8
B, I, H = 8192, 1024, 1024
K = I + H                      # 2048 contraction dim
BL = B // N_CORES              # 1024 batch rows per core
KC = K // 128                  # 16 K-chunks
QC = H // 128                  # 8 hidden chunks of 128
MC = 2                         # batch chunks of 512 per core
MT = BL // MC                  # 512
NCHUNKS = 4 * QC               # 32 (q-major, gate-minor) N-chunks of 128

_DT_MM = mybir.dt.float32r     # matmul operand dtype (fp32 bits, fast path)


def set_mm_dtype(name):
    """Switch matmul operand dtype ('fp32r' | 'bf16' | 'fp32'). Test-only."""
    global _DT_MM, _NP_MM
    import ml_dtypes
    _DT_MM = {"fp32r": mybir.dt.float32r, "bf16": mybir.dt.bfloat16,
              "fp32": mybir.dt.float32}[name]
    _NP_MM = ml_dtypes.bfloat16 if name == "bf16" else np.float32
    _NC_CACHE.clear()


_NP_MM = np.float32

# chain order within a group: f, i, C~ (tanh), o — o last so the final
# epilogue's critical path after the last matmul is just sigmoid(o)*tanh(C_t)
GATE_ORDER = (0, 1, 3, 2)

_SIG = mybir.ActivationFunctionType.Sigmoid
_TANH = mybir.ActivationFunctionType.Tanh


def build_program(repeats: int = 1):
    """Build the per-core Bass program. `repeats` unrolls the whole body
    (same data) for slope-based HW timing in test harnesses."""
    nc = bacc.Bacc("TRN2", target_bir_lowering=False, debug=False)

    # Host-prepped layouts (see prep_inputs):
    #   comb: [128, KC, BL]   combined^T, partition-major contiguous
    #   w:    [NCHUNKS, 128, KC, 128]  W tiles, partition-major contiguous
    #   bt:   [128, NCHUNKS]  bias chunks
    #   cp:   [128, QC, BL]   C_prev^T
    comb_d = nc.dram_tensor("comb", [128, KC, BL], _DT_MM, kind="ExternalInput")
    w_d = nc.dram_tensor("w", [NCHUNKS, 128, KC, 128], _DT_MM, kind="ExternalInput")
    bt_d = nc.dram_tensor("bt", [128, NCHUNKS], mybir.dt.float32, kind="ExternalInput")
    cp_d = nc.dram_tensor("cp", [128, QC, BL], mybir.dt.float32, kind="ExternalInput")
    ht_d = nc.dram_tensor("ht", [QC, 128, BL], mybir.dt.float32, kind="ExternalOutput")
    ct_d = nc.dram_tensor("ct", [QC, 128, BL], mybir.dt.float32, kind="ExternalOutput")

    with tile.TileContext(nc) as tc:
        with (
            tc.tile_pool(name="res", bufs=1) as res,
            tc.tile_pool(name="wp", bufs=8) as wp,
            tc.tile_pool(name="cpp", bufs=4) as cpp,
            tc.tile_pool(name="gp", bufs=2) as gp,
            tc.tile_pool(name="ep", bufs=2) as ep,
            tc.tile_pool(name="psum", bufs=2, space="PSUM") as pp,
        ):
            # q0's W tiles split into k-quarters so the first accumulation
            # chain starts after ~256KB instead of 1MB; combined^T split per
            # (K-chunk, m-half) for the same reason. GATE_ORDER puts the o
            # gate last so the final epilogue only waits on one activation.
            KSUB = 4
            # DMA emission ordered by first-use time: gate-0 W quarters, then
            # the m=0 combined chunks its chain consumes, then the remaining
            # gates' W, then the m=1 chunks.
            wts0 = [[] for _ in range(4)]
            cts = [[None] * MC for _ in range(KC)]

            def _load_wq0(g):
                for kq in range(KC // KSUB):
                    wt = wp.tile([128, KSUB, 128], _DT_MM, tag="wq0", name=f"wt0_{g}_{kq}")
                    nc.sync.dma_start(
                        out=wt[:], in_=w_d.ap()[GATE_ORDER[g], :, kq * KSUB : (kq + 1) * KSUB, :]
                    )
                    wts0[g].append(wt)

            def _load_ct(k, m):
                ctk = res.tile([128, MT], _DT_MM, tag=f"ct{k}_{m}", name=f"ct{k}_{m}")
                nc.sync.dma_start(
                    out=ctk[:], in_=comb_d.ap()[:, k, m * MT : (m + 1) * MT]
                )
                cts[k][m] = ctk

            _load_wq0(0)
            for k in range(KC):
                _load_ct(k, 0)
            _load_wq0(1)
            bt_sb = res.tile([128, NCHUNKS], mybir.dt.float32)
            nc.sync.dma_start(out=bt_sb[:], in_=bt_d.ap())
            _load_wq0(2)
            _load_wq0(3)
            for k in range(KC):
                for m in range(1, MC):
                    _load_ct(k, m)

            for _ in range(repeats):
                for q in range(QC):
                    if q == 0 and wts0 is not None:
                        wts = wts0
                        wts0 = None
                        ksub = KSUB
                    else:
                        ksub = KC
                        wts = []
                        for g in range(4):
                            c = q * 4 + GATE_ORDER[g]
                            wt = wp.tile([128, KC, 128], _DT_MM, tag="wt", name=f"wt{q}_{g}")
                            nc.sync.dma_start(out=wt[:], in_=w_d.ap()[c])
                            wts.append([wt])
                    for m in range(MC):
                        ms = slice(m * MT, (m + 1) * MT)
                        ps = [
                            pp.tile([128, MT], mybir.dt.float32, name=f"ps{g}", tag=f"ps{g}")
                            for g in range(4)
                        ]
                        # g-outer/k-inner: chain g completes after only its
                        # own W tile + the combined chunks, and its activation
                        # overlaps the remaining chains
                        for g in range(4):
                            for k in range(KC):
                                nc.tensor.matmul(
                                    ps[g][:],
                                    lhsT=wts[g][k // ksub][:, k % ksub, :],
                                    rhs=cts[k][m][:],
                                    start=(k == 0),
                                    stop=(k == KC - 1),
                                )
                        # epilogue: chains finish in order f,i,cl,o; o's
                        # sigmoid + final mul are the only ops after the last
                        # matmul of the group. cp load emitted after the MMs so
                        # W tiles keep DMA queue priority.
                        cp_t = cpp.tile([128, MT], mybir.dt.float32, tag="cp")
                        nc.sync.dma_start(out=cp_t[:], in_=cp_d.ap()[:, q, ms])
                        c0 = q * 4
                        f_sb = gp.tile([128, MT], mybir.dt.float32, tag="f", name="f_sb")
                        i_sb = gp.tile([128, MT], mybir.dt.float32, tag="i", name="i_sb")
                        o_sb = gp.tile([128, MT], mybir.dt.float32, tag="o", name="o_sb")
                        cl_sb = gp.tile([128, MT], mybir.dt.float32, tag="cl", name="cl_sb")
                        nc.scalar.activation(f_sb[:], ps[0][:], _SIG, bias=bt_sb[:, c0 : c0 + 1])
                        nc.scalar.activation(i_sb[:], ps[1][:], _SIG, bias=bt_sb[:, c0 + 1 : c0 + 2])
                        nc.scalar.activation(cl_sb[:], ps[2][:], _TANH, bias=bt_sb[:, c0 + 3 : c0 + 4])
                        # C_t = f*C_prev + i*ctilda ; h_t = o*tanh(C_t)
                        t1 = ep.tile([128, MT], mybir.dt.float32, tag="t1", name="t1")
                        t2 = ep.tile([128, MT], mybir.dt.float32, tag="t2", name="t2")
                        c_out = ep.tile([128, MT], mybir.dt.float32, tag="c_out", name="c_out")
                        th = ep.tile([128, MT], mybir.dt.float32, tag="th", name="th")
                        h_out = ep.tile([128, MT], mybir.dt.float32, tag="h_out", name="h_out")
                        nc.vector.tensor_tensor(
                            t1[:], f_sb[:], cp_t[:], mybir.AluOpType.mult
                        )
                        nc.vector.tensor_tensor(
                            t2[:], i_sb[:], cl_sb[:], mybir.AluOpType.mult
                        )
                        nc.vector.tensor_tensor(
                            c_out[:], t1[:], t2[:], mybir.AluOpType.add
                        )
                        nc.scalar.activation(th[:], c_out[:], _TANH)
                        nc.sync.dma_start(out=ct_d.ap()[q, :, ms], in_=c_out[:])
                        last = q == QC - 1 and m == MC - 1
                        if last:
                            # split the final o->h chain so ACT/DVE/DMA overlap
                            # after the very last matmul
                            hw_ = MT // 2
                            for s in range(2):
                                sl = slice(s * hw_, (s + 1) * hw_)
                                osl = slice(m * MT + s * hw_, m * MT + (s + 1) * hw_)
                                nc.scalar.activation(
                                    o_sb[:, sl], ps[3][:, sl], _SIG,
                                    bias=bt_sb[:, c0 + 2 : c0 + 3],
                                )
                                nc.vector.tensor_tensor(
                                    h_out[:, sl], o_sb[:, sl], th[:, sl],
                                    mybir.AluOpType.mult,
                                )
                                nc.sync.dma_start(
                                    out=ht_d.ap()[q, :, osl], in_=h_out[:, sl]
                                )
                        else:
                            nc.scalar.activation(o_sb[:], ps[3][:], _SIG, bias=bt_sb[:, c0 + 2 : c0 + 3])
                            nc.vector.tensor_tensor(
                                h_out[:], o_sb[:], th[:], mybir.AluOpType.mult
                            )
                            nc.sync.dma_start(out=ht_d.ap()[q, :, ms], in_=h_out[:])
    nc.compile()
    return nc


def prep_inputs(x, h_prev, C_prev, W_f, b_f, W_i, b_i, W_C, b_C, W_o, b_o):
    """Shard + lay out host arrays for the device program. Returns in_maps."""
    f32 = np.float32
    x = np.ascontiguousarray(x, f32)
    h_prev = np.ascontiguousarray(h_prev, f32)
    C_prev = np.ascontiguousarray(C_prev, f32)

    # W tiles: w5[c, p, ko, n] = W_gate[ko*128+p, q*128+n], c = q*4+g
    # Build as [QC, 4, 128(p), KC, 128(n)] then reshape.
    w5 = np.empty((QC, 4, 128, KC, 128), f32)
    for g, Wg in enumerate((W_f, W_i, W_o, W_C)):
        Wg = np.ascontiguousarray(Wg, f32)
        # [K, H] -> [KC, 128(p), QC, 128(n)] -> (q, p, ko, n)
        wr = Wg.reshape(KC, 128, QC, 128)
        w5[:, g] = wr.transpose(2, 1, 0, 3)
    w5 = np.ascontiguousarray(w5.reshape(NCHUNKS, 128, KC, 128).astype(_NP_MM))

    bt = np.empty((QC, 4, 128), f32)
    for g, bg in enumerate((b_f, b_i, b_o, b_C)):
        bt[:, g] = np.asarray(bg, f32).reshape(QC, 128)
    bt = np.ascontiguousarray(bt.reshape(NCHUNKS, 128).T)  # [128, NCHUNKS]

    in_maps = []
    for c in range(N_CORES):
        rs = slice(c * BL, (c + 1) * BL)
        # combined^T: [128(p), KC, BL]; rows 0..I-1 = x^T, I..K-1 = h^T
        comb = np.empty((KC, 128, BL), f32)
        comb.reshape(K, BL)[:I] = x[rs].T
        comb.reshape(K, BL)[I:] = h_prev[rs].T
        comb = np.ascontiguousarray(comb.transpose(1, 0, 2).astype(_NP_MM))
        # C_prev^T: [128(p), QC, BL]
        cp = np.ascontiguousarray(
            C_prev[rs].T.reshape(QC, 128, BL).transpose(1, 0, 2)
        )
        in_maps.append({"comb": comb, "w": w5, "bt": bt, "cp": cp})
    return in_maps


def assemble_outputs(results):
    """Gather per-core [QC, 128, BL] outputs into full [B, H] h_t, C_t."""
    h_t = np.empty((B, H), np.float32)
    C_t = np.empty((B, H), np.float32)
    for c, r in enumerate(results):
        rs = slice(c * BL, (c + 1) * BL)
        # [QC, 128, BL] -> [BL, QC*128]
        h_t[rs] = r["ht"].reshape(H, BL).T
        C_t[rs] = r["ct"].reshape(H, BL).T
    return h_t, C_t


_NC_CACHE = {}


def kernel(**inputs):
    if "nc" not in _NC_CACHE:
        _NC_CACHE["nc"] = build_program(repeats=1)
    nc = _NC_CACHE["nc"]
    in_maps = prep_inputs(**inputs)
    res = run_bass_kernel_spmd(nc, in_maps, core_ids=list(range(N_CORES)))
    return assemble_outputs(res.results)
